# revision 29
# baseline (speedup 1.0000x reference)
"""MixerBlock kernel for 8 Trainium2 NeuronCores.

Problem (hardcoded shapes): x (4, 1024, 2048) f32; per-head causal mixing.

  xt = x^T @ in_w.T + in_b                      # (B, S, E)
  p  = heads(xt)                                # (B, H, e, S), c = h*64+e
  col heads h<8:  out[c,s] = v_h[s] * sum_{t<=s} p[c,t]
  row heads h>=8: out[c,s] = sum_{t<=s} v_h[t] * p[c,t]
  out = merge @ out_w.T + out_b, transposed back to (B, E, S)

The S x S mixing matrices are rank-structured causal, so the einsum collapses
to a cumulative sum along t with a per-head pre-scale (row heads) or
post-scale (col heads).

Sharding: 8 cores = (batch b in 0..3) x (head-group g in {0,1}).  Group 0 =
col heads (channels 0..511), group 1 = row heads (channels 512..1023).  Each
core computes in_proj for its 512 channels, the cumsum mixing (vector-engine
tensor_tensor_scan along the free dim), and a partial out_proj over its
channel slice, producing a full-size (E, S) partial output.  Host sums the
two partials per batch.  No cross-core communication.  The pre/post scales
are unified SPMD-style: each core receives both a "pre" and "post" (8, S)
array, one of which is all-ones depending on its head-group type.

Biases (all zero in setup_inputs) enter linearly and are folded in on the
host via a closed form when nonzero.
"""

import os

import numpy as np

B, E, S, H = 4, 1024, 2048, 16
C = 512          # channels per core (8 heads x 64)
P = 128
NK = E // P      # 8 contraction tiles for in_proj
NM = C // P      # 4 local-channel tiles
NHALF = 2        # t halves for x streaming
TQ = S // NHALF  # 1024
NQ = TQ // 512   # 2 512-chunks per half
NEO = E // P     # 8 output-row tiles
NS = S // 512    # 4 512-wide s slices
N_CORES = 8

_NC = None


def _build_nc(mode="bf16"):
    from contextlib import ExitStack

    import concourse.bacc as bacc
    import concourse.mybir as mybir
    import concourse.tile as tile
    from concourse.alu_op_type import AluOpType
    from concourse.tile import add_dep_helper

    f32 = mybir.dt.float32
    mm_dt = {"bf16": mybir.dt.bfloat16,
             "f32r": mybir.dt.float32r,
             "f32": f32}[mode]
    io_dt = mybir.dt.bfloat16 if mode == "bf16" else f32

    nc = bacc.Bacc(
        "TRN2",
        target_bir_lowering=False,
        debug=False,
        enable_asserts=True,
        num_devices=N_CORES,
    )
    x_d = nc.dram_tensor("x", (E, S), mm_dt, kind="ExternalInput").ap()
    win_d = nc.dram_tensor("w_in", (E, C), mm_dt, kind="ExternalInput").ap()
    wout_d = nc.dram_tensor("w_out", (C, E), mm_dt, kind="ExternalInput").ap()
    # pre/post arrive pre-expanded to one row per channel (host-side
    # repeat): plain large HWDGE loads instead of 32 serialized SWDGE
    # partition-broadcast DMAs
    pre_d = nc.dram_tensor("pre", (C, S), io_dt, kind="ExternalInput").ap()
    post_d = nc.dram_tensor("post", (C, S), io_dt, kind="ExternalInput").ap()
    out_d = nc.dram_tensor("out", (E, S), io_dt, kind="ExternalOutput").ap()

    xt = x_d.rearrange("(ko p) t -> p ko t", p=P)        # (128, 8, 2048)
    wi = win_d.rearrange("(ko p) c -> p ko c", p=P)      # (128, 8, 512)
    wo = wout_d.rearrange("(kc p) eo -> p kc eo", p=P)   # (128, 4, 1024)
    prer = pre_d.rearrange("(m p) s -> p m s", p=P)      # (128, 4, 2048)
    postr = post_d.rearrange("(m p) s -> p m s", p=P)    # (128, 4, 2048)
    outr = out_d.rearrange("(mo p) s -> p mo s", p=P)    # (128, 8, 2048)

    with tile.TileContext(nc) as tc:
        with ExitStack() as ctx:
            wpool = ctx.enter_context(tc.tile_pool(name="w", bufs=1))
            xpool = ctx.enter_context(tc.tile_pool(name="xc", bufs=1))
            scpool = ctx.enter_context(tc.tile_pool(name="sc", bufs=1))
            qpool = ctx.enter_context(tc.tile_pool(name="q", bufs=1))
            mixpool = ctx.enter_context(tc.tile_pool(name="mix", bufs=1))
            outpool = ctx.enter_context(tc.tile_pool(name="o", bufs=2))
            pp = ctx.enter_context(tc.tile_pool(name="pp", bufs=8, space="PSUM"))

            w_in_sb = wpool.tile([P, NK, C], mm_dt, tag="w_in")
            w_out_sb = wpool.tile([P, NM, E], mm_dt, tag="w_out")

            xh = []
            for h in range(NHALF):
                t = xpool.tile([P, NK, TQ], mm_dt, tag=f"xh{h}", name=f"xh{h}")
                xh.append(t)
            pre_sb = scpool.tile([P, NM, S], io_dt, tag="pre")
            post_sb = scpool.tile([P, NM, S], io_dt, tag="post")
            # single FIFO HWDGE stream on the sync ring, ordered by
            # first-need time of each chunk (PE consumption + DVE chain)
            h0, h1 = slice(0, TQ), slice(TQ, S)
            # NOTE: keep the whole input stream on ONE ring (sync): a
            # second HWDGE ring shares the 16 SDMA engines at packet
            # granularity and stalls the rate-matched x stream
            nc.sync.dma_start(w_in_sb[:, 0:2, :], wi[:, 0:2, :])
            nc.sync.dma_start(xh[0][:, 0:2, 0:512], xt[:, 0:2, 0:512])
            nc.sync.dma_start(xh[0][:, 0:2, 512:TQ], xt[:, 0:2, 512:TQ])
            nc.sync.dma_start(w_in_sb[:, 2:8, :], wi[:, 2:8, :])
            nc.sync.dma_start(xh[0][:, 2:4, :], xt[:, 2:4, h0])
            nc.sync.dma_start(xh[0][:, 4:6, :], xt[:, 4:6, h0])
            nc.sync.dma_start(pre_sb[:, 0:2, h0], prer[:, 0:2, h0])
            nc.sync.dma_start(xh[0][:, 6:8, :], xt[:, 6:8, h0])
            nc.sync.dma_start(pre_sb[:, 2:4, h0], prer[:, 2:4, h0])
            nc.sync.dma_start(post_sb[:, :, h0], postr[:, :, h0])
            nc.sync.dma_start(xh[1][:, 0:2, :], xt[:, 0:2, h1])
            nc.sync.dma_start(xh[1][:, 2:4, :], xt[:, 2:4, h1])
            nc.sync.dma_start(xh[1][:, 4:6, :], xt[:, 4:6, h1])
            nc.sync.dma_start(pre_sb[:, :, h1], prer[:, :, h1])
            nc.sync.dma_start(xh[1][:, 6:8, :], xt[:, 6:8, h1])
            nc.sync.dma_start(w_out_sb[:, 0:2, :], wo[:, 0:2, :])
            nc.sync.dma_start(post_sb[:, :, h1], postr[:, :, h1])
            nc.sync.dma_start(w_out_sb[:, 2:4, :], wo[:, 2:4, :])

            # HAM warm-up: dummy matmuls on a zeroed scratch tile keep the
            # PE busy through the ~10us DMA head so the real matmul stream
            # starts at full clock (K=8/8)
            scratch = scpool.tile([P, 640], mm_dt, tag="warm")
            nc.gpsimd.memzero(scratch[:])

            # q/cum in bf16: scan state stays fp32 internally, and the
            # all-bf16 post-multiply hits the DVE 2x packed mode
            q = [qpool.tile([P, S], io_dt, tag=f"q{m}", name=f"q{m}")
                 for m in range(NM)]
            mixed = [mixpool.tile([P, S], mm_dt, tag=f"mx{m}", name=f"mx{m}")
                     for m in range(NM)]
            # cum tiles stay alive across halves: the h1 scan seeds from
            # cum_h0[:, -1] directly (no boundary stash / copy op)
            cumpool = ctx.enter_context(tc.tile_pool(name="cum", bufs=8))
            cums = {}

            # ---- in_proj + pre-scale + chained scan/post ----
            for h in range(NHALF):
                ps = {(m, n): pp.tile([P, 512], f32, tag="ps",
                                      name=f"pp{h}_{m}_{n}")
                      for m in range(NM) for n in range(NQ)}
                if h == 0:
                    # warm-up dummies overwrite ps[(0,0)] (start+stop per
                    # matmul, result never read) while the x head streams
                    for _ in range(12):
                        nc.tensor.matmul(
                            ps[(0, 0)][:],
                            lhsT=scratch[:, 0:128],
                            rhs=scratch[:, 128:640],
                            start=True, stop=True,
                        )
                # hybrid order: ko 0-3 k-outer (PE keeps pace with the x
                # stream), then m-outer over ko 4-7 so m0's psums complete
                # ~6us after the half starts and the DVE chain can begin
                # while the PE is still on m1-m3.  The first two ko passes
                # split by n-half so the PE's early consumption rate stays
                # below the just-started x stream's delivery rate.
                loop = [(ko, m, n) for n in range(NQ)
                        for ko in range(2) for m in range(NM)]
                loop += [(ko, m, n) for ko in range(2, 4)
                         for m in range(NM) for n in range(NQ)]
                loop += [(ko, m, n) for m in range(NM)
                         for ko in range(4, NK) for n in range(NQ)]
                for ko, m, n in loop:
                    nc.tensor.matmul(
                        ps[(m, n)][:],
                        lhsT=w_in_sb[:, ko, m * P:(m + 1) * P],
                        rhs=xh[h][:, ko, n * 512:(n + 1) * 512],
                        start=(ko == 0),
                        stop=(ko == NK - 1),
                    )
                lo = h * TQ
                # all pre-scales first (DVE): frees the psum banks as fast
                # as the DVE can drain them, so the next half / out_proj
                # never stalls on PSUM reuse
                for m in range(NM):
                    for n in range(NQ):
                        so = lo + n * 512
                        nc.vector.tensor_tensor(
                            q[m][:, so:so + 512],
                            ps[(m, n)][:],
                            pre_sb[:, m, so:so + 512],
                            AluOpType.mult,
                        )
                # causal mixing: chained cumsum + post-scale, all DVE (any
                # GpSimd SBUF activity disables the DVE 2-port mode and
                # taxes the scans ~30%)
                for m in range(NM):
                    qh = q[m][:, lo:lo + TQ]
                    cum_t = cumpool.tile([P, TQ], io_dt, tag="cum",
                                         name=f"cum{m}_{h}")
                    cums[(h, m)] = cum_t
                    init = 0.0 if h == 0 else cums[(0, m)][:, TQ - 1:TQ]
                    nc.vector.tensor_tensor_scan(
                        cum_t[:], qh, qh, init,
                        AluOpType.add, AluOpType.bypass,
                    )
                    nc.vector.tensor_tensor(
                        mixed[m][:, lo:lo + TQ], cum_t[:],
                        post_sb[:, m, lo:lo + TQ],
                        AluOpType.mult,
                    )

            # ---- out_proj (partial over this core's 512 channels) ----
            # s-outer: s slices 0/1 (first-half mixed, ready early) run
            # while the DVE mixing chain still produces the second half.
            # Copies alternate Scalar/Vector; out DMAs are batched 4 tiles
            # per transfer on the ACT ring (off the x input ring).
            for so in range(NS):
                ot = outpool.tile([P, NEO, 512], io_dt, tag="o",
                                  name=f"o{so}")
                for mo in range(NEO):
                    pt = pp.tile([P, 512], f32, tag="ps", name=f"po{so}_{mo}")
                    for kc in range(NM):
                        nc.tensor.matmul(
                            pt[:],
                            lhsT=w_out_sb[:, kc, mo * P:(mo + 1) * P],
                            rhs=mixed[kc][:, so * 512:(so + 1) * 512],
                            start=(kc == 0),
                            stop=(kc == NM - 1),
                        )
                    if mo % 2 == 0:
                        nc.scalar.copy(out=ot[:, mo, :], in_=pt[:])
                    else:
                        nc.vector.tensor_copy(out=ot[:, mo, :], in_=pt[:])
                    # out DMAs on the (idle by now) sync engine; the last
                    # s-slice goes per-tile with triggers split across
                    # sync/scalar so the final transfer issues immediately
                    ng = 2 if so < NS - 1 else 8
                    step = NEO // ng
                    if mo % step == step - 1:
                        g0 = mo + 1 - step
                        dma_eng = nc.sync
                        if so == NS - 1 and mo % 2 == 0:
                            dma_eng = nc.scalar
                        dma_eng.dma_start(
                            outr[:, g0:mo + 1, so * 512:(so + 1) * 512],
                            ot[:, g0:mo + 1, :])
    nc.compile()
    return nc


def _mode():
    return os.environ.get("MIXER_DTYPE", "bf16")


def _get_nc():
    global _NC
    if _NC is None:
        _NC = _build_nc(mode=_mode())
    return _NC


def shard_inputs(x, in_w, out_w, mix_w):
    if _mode() == "bf16":
        import ml_dtypes

        dt = ml_dtypes.bfloat16
    else:
        dt = np.float32
    x = np.ascontiguousarray(np.asarray(x, np.float32).astype(dt))
    in_w = np.asarray(in_w, np.float32)
    out_w = np.asarray(out_w, np.float32)
    mix_w = np.ascontiguousarray(mix_w, np.float32)
    ones = np.ones((8, S), np.float32)
    group = []
    for g in range(2):
        cs = slice(g * C, (g + 1) * C)
        pre = ones if g == 0 else mix_w[8:16]
        post = mix_w[0:8] if g == 0 else ones
        group.append({
            "w_in": np.ascontiguousarray(in_w[cs, :].T.astype(dt)),
            "w_out": np.ascontiguousarray(out_w[:, cs].T.astype(dt)),
            # expand to one row per channel (head h covers channels
            # 64h..64h+63 of this core's 512)
            "pre": np.ascontiguousarray(np.repeat(pre, 64, axis=0).astype(dt)),
            "post": np.ascontiguousarray(
                np.repeat(post, 64, axis=0).astype(dt)),
        })
    in_maps = []
    for b in range(B):
        for g in range(2):
            m = {"x": x[b]}
            m.update(group[g])
            in_maps.append(m)
    return in_maps


def _bias_contribution(in_b, out_b, mix_b, mix_w, out_w):
    """Closed-form (E, S) addend from the (linear) bias terms."""
    if not (np.any(in_b) or np.any(out_b) or np.any(mix_b)):
        return None
    s_idx = np.arange(S, dtype=np.float64)
    bias1 = np.zeros((E, S), np.float64)
    for h in range(H):
        cs = slice(h * 64, (h + 1) * 64)
        v = np.asarray(mix_w[h], np.float64)
        if h < H // 2:
            g = (s_idx + 1.0) * v          # cumsum of constant, then *v[s]
        else:
            g = np.cumsum(v)               # cumsum of v[t]
        bias1[cs] = np.asarray(in_b, np.float64)[cs, None] * g[None, :]
        bias1[cs] += np.asarray(mix_b[h], np.float64)[None, :]
    fb = np.asarray(out_w, np.float64) @ bias1
    fb += np.asarray(out_b, np.float64)[:, None]
    return fb.astype(np.float32)


def run_sharded(in_maps, trace=False):
    from concourse.bass_utils import run_bass_kernel_spmd

    return run_bass_kernel_spmd(
        _get_nc(), in_maps, core_ids=list(range(N_CORES)), trace=trace
    )


def gather_output(results, bias_fb=None):
    out = np.empty((B, E, S), np.float32)
    for b in range(B):
        out[b] = (np.asarray(results[2 * b]["out"], np.float32)
                  + np.asarray(results[2 * b + 1]["out"], np.float32))
        if bias_fb is not None:
            out[b] += bias_fb
    return out


def kernel(x, in_w, in_b, out_w, out_b, mix_w, mix_b):
    in_maps = shard_inputs(x, in_w, out_w, mix_w)
    res = run_sharded(in_maps, trace=False)
    fb = _bias_contribution(
        np.asarray(in_b), np.asarray(out_b), np.asarray(mix_b),
        np.asarray(mix_w), np.asarray(out_w))
    return gather_output(res.results, fb)

